# revision 3
# baseline (speedup 1.0000x reference)
# Adaptive softmax (head 2002 + tail0 8000 + tail1 40000 -> [4096, 50000] log-probs)
# on 8 TRN2 NeuronCores, data-parallel over the 4096 tokens (512 tokens/core).
#
# Per core: all matmuls run in fp16 on the TensorEngine (weights pre-transposed
# and cast on host), log-softmax statistics accumulated with the ScalarEngine's
# Exp+accum_out, and the final corrected f32 rows DMA'd straight to the output.
# head/tail0 logits are materialized in SBUF (fp16) so their weights stream
# once; tail1 (the 40000-wide projection) is computed twice (sumexp pass +
# output pass) to avoid materializing 20MB of logits, trading PE flops for
# HBM traffic.
import os
import sys

for _p in (
    "/root/.axon_site",
    "/root/.axon_site/_ro/trn_rl_repo",
    "/root/.axon_site/_ro/pypackages",
    "/opt/trn_rl_repo",
    "/opt/pypackages",
):
    if os.path.isdir(_p) and _p not in sys.path:
        sys.path.append(_p)

import numpy as np

import concourse.bass as bass
import concourse.mybir as mybir
import concourse.tile as tile
from concourse import bacc
from concourse.bass_utils import run_bass_kernel_spmd

B = 4096  # tokens total
D = 1024  # hidden
NCORES = 8
T = B // NCORES  # 512 tokens per core
MCH = T // 128  # 4 token chunks of 128
KD = D // 128  # 8 k-tiles for D
OUT_HEAD = 2002
C0 = 2000
V0 = 8000  # tail0 vocab width
V1 = 40000  # tail1 vocab width
H1 = 256  # tail1 reduced hidden
K1 = H1 // 128  # 2
C2 = 50000
T0_OFF = 2000  # output column offset of tail0 block
T1_OFF = 10000  # output column offset of tail1 block

F16 = mybir.dt.float16
F32 = mybir.dt.float32
AF = mybir.ActivationFunctionType
ALU = mybir.AluOpType
X_AXIS = mybir.AxisListType.X


def _blocks(width, bw):
    return [(o, min(bw, width - o)) for o in range(0, width, bw)]


def _r(ap):
    # DRAM [K, N] viewed as [p, a, n] so one DMA loads all K-tiles of a column block
    return ap.rearrange("(a p) n -> p a n", p=128)


def build():
    nc = bacc.Bacc(None, target_bir_lowering=False)
    xT = nc.declare_dram_parameter("xT", [D, T], F16, isOutput=False)
    WhT = nc.declare_dram_parameter("WhT", [D, OUT_HEAD], F16, isOutput=False)
    bh = nc.declare_dram_parameter("bh", [1, OUT_HEAD], F16, isOutput=False)
    W0aT = nc.declare_dram_parameter("W0aT", [D, D], F16, isOutput=False)
    W0bT = nc.declare_dram_parameter("W0bT", [D, V0], F16, isOutput=False)
    W1aT = nc.declare_dram_parameter("W1aT", [D, H1], F16, isOutput=False)
    W1bT = nc.declare_dram_parameter("W1bT", [H1, V1], F16, isOutput=False)
    out = nc.declare_dram_parameter("out", [T, C2], F32, isOutput=True)

    n_vt1 = (V1 + 511) // 512  # 79 global 512-wide tiles of tail1

    with tile.TileContext(nc) as tc:
        with (
            tc.tile_pool(name="const", bufs=1) as cpool,
            tc.tile_pool(name="logits", bufs=1) as lpool,
            tc.tile_pool(name="stats", bufs=1) as spool,
            tc.tile_pool(name="wblk", bufs=2) as wpool,
            tc.tile_pool(name="wblk1", bufs=2) as w1pool,
            tc.tile_pool(name="scr", bufs=3) as scpool,
            tc.tile_pool(name="stage", bufs=2) as stpool,
            tc.tile_pool(name="psum", bufs=6, space=bass.MemorySpace.PSUM) as ppool,
        ):
            # ---- resident inputs -------------------------------------------------
            xT_sb = cpool.tile([128, KD, T], F16)
            nc.sync.dma_start(out=xT_sb[:], in_=_r(xT[:]))
            w0a_sb = cpool.tile([128, KD, D], F16)
            nc.sync.dma_start(out=w0a_sb[:], in_=_r(W0aT[:]))
            w1a_sb = cpool.tile([128, KD, H1], F16)
            nc.sync.dma_start(out=w1a_sb[:], in_=_r(W1aT[:]))
            bh_sb = cpool.tile([1, OUT_HEAD], F16)
            nc.sync.dma_start(out=bh_sb[:], in_=bh[:])
            ones_sb = cpool.tile([1, 128], F16)
            nc.vector.memset(ones_sb[:], 1.0)

            h0T = cpool.tile([128, KD, T], F16)  # (x @ W0a.T).T, hid on partitions
            h1T = cpool.tile([128, K1, T], F16)  # (x @ W1a.T).T

            # ---- stats -----------------------------------------------------------
            head_sums = spool.tile([128, MCH, 4], F32)
            t0_sums = spool.tile([128, MCH, (V0 + 511) // 512], F32)
            t1_sums = spool.tile([128, MCH, n_vt1], F32)
            se_head = spool.tile([128, MCH], F32)
            se_t0 = spool.tile([128, MCH], F32)
            se_t1 = spool.tile([128, MCH], F32)
            lse_head = spool.tile([128, MCH], F32)
            lse_t0 = spool.tile([128, MCH], F32)
            lse_t1 = spool.tile([128, MCH], F32)
            c01 = spool.tile([128, MCH, 2], F32)  # head cluster logits (f32)
            neg_head = spool.tile([128, MCH], F32)
            tmp0 = spool.tile([128, MCH], F32)
            tmp1 = spool.tile([128, MCH], F32)
            neg0 = spool.tile([128, MCH], F32)
            neg1 = spool.tile([128, MCH], F32)

            # ---- phase H: hidden projections h0T / h1T ---------------------------
            for dst, wsb, nchunk in ((h0T, w0a_sb, KD), (h1T, w1a_sb, K1)):
                for hc in range(nchunk):
                    ps = ppool.tile([128, 512], F32)
                    for k in range(KD):
                        nc.tensor.matmul(
                            ps[:],
                            wsb[:, k, hc * 128 : (hc + 1) * 128],
                            xT_sb[:, k, :],
                            start=(k == 0),
                            stop=(k == KD - 1),
                        )
                    nc.vector.tensor_copy(dst[:, hc, :], ps[:])

            # ---- helper: one section of streamed matmul --------------------------
            def stream_section(wdram, width, kt, lhsT_sb, pool, blkw, visit):
                """for each column block of wdram: load once, then for each
                (token chunk m, 512-wide vtile) accumulate K matmuls into psum
                and hand psum to visit(m, global_off, vw, ps)."""
                for bo, bw in _blocks(width, blkw):
                    wb = pool.tile([128, kt, blkw], F16)
                    nc.sync.dma_start(
                        out=wb[:, :, :bw], in_=_r(wdram[:])[:, :, bo : bo + bw]
                    )
                    for m in range(MCH):
                        ms = slice(m * 128, (m + 1) * 128)
                        for vo, vw in _blocks(bw, 512):
                            ps = ppool.tile([128, 512], F32)
                            for k in range(kt):
                                nc.tensor.matmul(
                                    ps[:, :vw],
                                    lhsT_sb[:, k, ms],
                                    wb[:, k, vo : vo + vw],
                                    start=(k == 0),
                                    stop=False,
                                )
                            visit(m, bo + vo, vw, ps)

            # ---- HEAD section (materialize logits fp16, exp-accumulate) ----------
            head_logits = lpool.tile([128, MCH, OUT_HEAD], F16)

            def head_visit(m, go, vw, ps):
                # fold in bh via a K=1 matmul of ones.T @ bh, closing the group
                nc.tensor.matmul(
                    ps[:, :vw],
                    ones_sb[:, :],
                    bh_sb[:, go : go + vw],
                    start=False,
                    stop=True,
                )
                nc.vector.tensor_copy(head_logits[:, m, go : go + vw], ps[:, :vw])
                if go + vw == OUT_HEAD:
                    nc.vector.tensor_copy(c01[:, m, :], ps[:, vw - 2 : vw])
                sc = scpool.tile([128, 512], F16, tag="expsc")
                gi = go // 512
                nc.scalar.activation(
                    sc[:, :vw],
                    ps[:, :vw],
                    AF.Exp,
                    accum_out=head_sums[:, m, gi : gi + 1],
                )

            stream_section(WhT, OUT_HEAD, KD, xT_sb, wpool, 1024, head_visit)

            for m in range(MCH):
                nc.vector.tensor_reduce(
                    se_head[:, m : m + 1], head_sums[:, m, :], X_AXIS, ALU.add
                )
                nc.scalar.activation(lse_head[:, m : m + 1], se_head[:, m : m + 1], AF.Ln)
                nc.vector.tensor_scalar_mul(
                    neg_head[:, m : m + 1], lse_head[:, m : m + 1], -1.0
                )

            # head output rows: out[:, :2000] = head_logits - lse_head
            for m in range(MCH):
                ms = slice(m * 128, (m + 1) * 128)
                st = stpool.tile([128, 2048], F32, tag="stage")
                nc.scalar.activation(
                    st[:, :C0],
                    head_logits[:, m, :C0],
                    AF.Identity,
                    bias=neg_head[:, m : m + 1],
                )
                nc.sync.dma_start(out=out[ms, 0:C0], in_=st[:, :C0])

            # ---- TAIL0 section ---------------------------------------------------
            t0_logits = lpool.tile([128, MCH, V0], F16)

            def t0_section():
                for bo, bw in _blocks(V0, 1024):
                    wb = wpool.tile([128, KD, 1024], F16)
                    nc.sync.dma_start(
                        out=wb[:, :, :bw], in_=_r(W0bT[:])[:, :, bo : bo + bw]
                    )
                    for m in range(MCH):
                        ms = slice(m * 128, (m + 1) * 128)
                        for vo, vw in _blocks(bw, 512):
                            go = bo + vo
                            ps = ppool.tile([128, 512], F32)
                            for k in range(KD):
                                nc.tensor.matmul(
                                    ps[:, :vw],
                                    h0T[:, k, ms],
                                    wb[:, k, vo : vo + vw],
                                    start=(k == 0),
                                    stop=(k == KD - 1),
                                )
                            nc.vector.tensor_copy(
                                t0_logits[:, m, go : go + vw], ps[:, :vw]
                            )
                            sc = scpool.tile([128, 512], F16, tag="expsc")
                            gi = go // 512
                            nc.scalar.activation(
                                sc[:, :vw],
                                ps[:, :vw],
                                AF.Exp,
                                accum_out=t0_sums[:, m, gi : gi + 1],
                            )

            t0_section()

            for m in range(MCH):
                nc.vector.tensor_reduce(
                    se_t0[:, m : m + 1], t0_sums[:, m, :], X_AXIS, ALU.add
                )
                nc.scalar.activation(lse_t0[:, m : m + 1], se_t0[:, m : m + 1], AF.Ln)
                # bias = c0 - lse_head - lse_t0
                nc.vector.tensor_sub(
                    tmp0[:, m : m + 1], c01[:, m, 0:1], lse_head[:, m : m + 1]
                )
                nc.vector.tensor_sub(
                    neg0[:, m : m + 1], tmp0[:, m : m + 1], lse_t0[:, m : m + 1]
                )

            for m in range(MCH):
                ms = slice(m * 128, (m + 1) * 128)
                for so, sw in _blocks(V0, 2048):
                    st = stpool.tile([128, 2048], F32, tag="stage")
                    nc.scalar.activation(
                        st[:, :sw],
                        t0_logits[:, m, so : so + sw],
                        AF.Identity,
                        bias=neg0[:, m : m + 1],
                    )
                    nc.sync.dma_start(
                        out=out[ms, T0_OFF + so : T0_OFF + so + sw], in_=st[:, :sw]
                    )

            # ---- TAIL1 pass 1: sumexp only (logits discarded) --------------------
            t1_blocks = _blocks(V1, 2048)

            def t1_matmuls(m, wb, vo, vw, ps):
                ms = slice(m * 128, (m + 1) * 128)
                for k in range(K1):
                    nc.tensor.matmul(
                        ps[:, :vw],
                        h1T[:, k, ms],
                        wb[:, k, vo : vo + vw],
                        start=(k == 0),
                        stop=(k == K1 - 1),
                    )

            for bo, bw in t1_blocks:
                wb = w1pool.tile([128, K1, 2048], F16)
                nc.sync.dma_start(
                    out=wb[:, :, :bw], in_=_r(W1bT[:])[:, :, bo : bo + bw]
                )
                for m in range(MCH):
                    for vo, vw in _blocks(bw, 512):
                        ps = ppool.tile([128, 512], F32)
                        t1_matmuls(m, wb, vo, vw, ps)
                        sc = scpool.tile([128, 512], F16, tag="expsc")
                        gi = (bo + vo) // 512
                        nc.scalar.activation(
                            sc[:, :vw],
                            ps[:, :vw],
                            AF.Exp,
                            accum_out=t1_sums[:, m, gi : gi + 1],
                        )

            for m in range(MCH):
                nc.vector.tensor_reduce(
                    se_t1[:, m : m + 1], t1_sums[:, m, :], X_AXIS, ALU.add
                )
                nc.scalar.activation(lse_t1[:, m : m + 1], se_t1[:, m : m + 1], AF.Ln)
                # bias = c1 - lse_head - lse_t1
                nc.vector.tensor_sub(
                    tmp1[:, m : m + 1], c01[:, m, 1:2], lse_head[:, m : m + 1]
                )
                nc.vector.tensor_sub(
                    neg1[:, m : m + 1], tmp1[:, m : m + 1], lse_t1[:, m : m + 1]
                )

            # ---- TAIL1 pass 2: recompute logits, correct, write ------------------
            for bo, bw in t1_blocks:
                wb = w1pool.tile([128, K1, 2048], F16)
                nc.sync.dma_start(
                    out=wb[:, :, :bw], in_=_r(W1bT[:])[:, :, bo : bo + bw]
                )
                for m in range(MCH):
                    ms = slice(m * 128, (m + 1) * 128)
                    st = stpool.tile([128, 2048], F32, tag="stage")
                    for vo, vw in _blocks(bw, 512):
                        ps = ppool.tile([128, 512], F32)
                        t1_matmuls(m, wb, vo, vw, ps)
                        nc.vector.tensor_scalar_add(
                            st[:, vo : vo + vw], ps[:, :vw], neg1[:, m : m + 1]
                        )
                    nc.sync.dma_start(
                        out=out[ms, T1_OFF + bo : T1_OFF + bo + bw], in_=st[:, :bw]
                    )

    nc.compile()
    return nc


_NC_CACHE = {}


def _get_nc():
    if "nc" not in _NC_CACHE:
        _NC_CACHE["nc"] = build()
    return _NC_CACHE["nc"]


def _prep_weights(Wh, bh, W0a, W0b, W1a, W1b):
    f = np.float16
    return {
        "WhT": np.ascontiguousarray(np.asarray(Wh, np.float32).T).astype(f),
        "bh": np.asarray(bh, np.float32).reshape(1, OUT_HEAD).astype(f),
        "W0aT": np.ascontiguousarray(np.asarray(W0a, np.float32).T).astype(f),
        "W0bT": np.ascontiguousarray(np.asarray(W0b, np.float32).T).astype(f),
        "W1aT": np.ascontiguousarray(np.asarray(W1a, np.float32).T).astype(f),
        "W1bT": np.ascontiguousarray(np.asarray(W1b, np.float32).T).astype(f),
    }


def kernel(x, Wh, bh, W0a, W0b, W1a, W1b, _trace=False):
    x = np.asarray(x, np.float32)
    nc = _get_nc()
    shared = _prep_weights(Wh, bh, W0a, W0b, W1a, W1b)
    in_maps = []
    for i in range(NCORES):
        m = dict(shared)
        m["xT"] = np.ascontiguousarray(x[i * T : (i + 1) * T].T).astype(np.float16)
        in_maps.append(m)
    res = run_bass_kernel_spmd(nc, in_maps, core_ids=list(range(NCORES)), trace=_trace)
    out = np.concatenate([res.results[i]["out"] for i in range(NCORES)], axis=0)
    if _trace:
        return out, res
    return out


# revision 6
# speedup vs baseline: 1.0494x; 1.0494x over previous
# Adaptive softmax (head 2002 + tail0 8000 + tail1 40000 -> [4096, 50000] log-probs)
# on 8 TRN2 NeuronCores, data-parallel over the 4096 tokens (512 tokens/core).
#
# Per core: all matmuls run in bf16 on the TensorEngine (weights pre-transposed
# and cast on host), log-softmax statistics accumulated with the ScalarEngine's
# Exp+accum_out on wide tiles, and the corrected f32 rows DMA'd straight to the
# output. head/tail0 logits are materialized in SBUF (fp16) so their weights
# stream once; tail1 (the 40000-wide projection) is computed twice (sumexp pass
# + output pass) to avoid materializing 20MB of logits, trading PE flops for
# HBM traffic.
import os
import sys

for _p in (
    "/root/.axon_site",
    "/root/.axon_site/_ro/trn_rl_repo",
    "/root/.axon_site/_ro/pypackages",
    "/opt/trn_rl_repo",
    "/opt/pypackages",
):
    if os.path.isdir(_p) and _p not in sys.path:
        sys.path.append(_p)

import ml_dtypes
import numpy as np

import concourse.bass as bass
import concourse.mybir as mybir
import concourse.tile as tile
from concourse import bacc
from concourse.bass_utils import run_bass_kernel_spmd

B = 4096  # tokens total
D = 1024  # hidden
NCORES = 8
T = B // NCORES  # 512 tokens per core
MCH = T // 128  # 4 token chunks of 128
KD = D // 128  # 8 k-tiles for D
OUT_HEAD = 2002
C0 = 2000
V0 = 8000  # tail0 vocab width
V1 = 40000  # tail1 vocab width
H1 = 256  # tail1 reduced hidden
K1 = H1 // 128  # 2
C2 = 50000
T0_OFF = 2000  # output column offset of tail0 block
T1_OFF = 10000  # output column offset of tail1 block

BF16 = mybir.dt.bfloat16
F16 = mybir.dt.float16
F32 = mybir.dt.float32
AF = mybir.ActivationFunctionType
ALU = mybir.AluOpType
X_AXIS = mybir.AxisListType.X


def _blocks(width, bw):
    return [(o, min(bw, width - o)) for o in range(0, width, bw)]


def _r(ap):
    # DRAM [K, N] viewed as [p, a, n] so one DMA loads all K-tiles of a column block
    return ap.rearrange("(a p) n -> p a n", p=128)


def build():
    nc = bacc.Bacc(None, target_bir_lowering=False)
    xT = nc.declare_dram_parameter("xT", [D, T], BF16, isOutput=False)
    WhT = nc.declare_dram_parameter("WhT", [D, OUT_HEAD], BF16, isOutput=False)
    bh = nc.declare_dram_parameter("bh", [1, OUT_HEAD], BF16, isOutput=False)
    W0aT = nc.declare_dram_parameter("W0aT", [D, D], BF16, isOutput=False)
    W0bT = nc.declare_dram_parameter("W0bT", [D, V0], BF16, isOutput=False)
    W1aT = nc.declare_dram_parameter("W1aT", [D, H1], BF16, isOutput=False)
    W1bT = nc.declare_dram_parameter("W1bT", [H1, V1], BF16, isOutput=False)
    out = nc.declare_dram_parameter("out", [T, C2], F32, isOutput=True)

    t1_blocks = _blocks(V1, 2048)  # 20 streamed blocks for tail1
    n_t1b = len(t1_blocks)

    with tile.TileContext(nc) as tc:
        with (
            tc.tile_pool(name="const", bufs=1) as cpool,
            tc.tile_pool(name="logits", bufs=1) as lpool,
            tc.tile_pool(name="stats", bufs=1) as spool,
            tc.tile_pool(name="wblk", bufs=2) as wpool,
            tc.tile_pool(name="wblk1", bufs=2) as w1pool,
            tc.tile_pool(name="scr", bufs=3) as scpool,
            tc.tile_pool(name="stage", bufs=2) as stpool,
            tc.tile_pool(name="psum", bufs=2, space=bass.MemorySpace.PSUM) as ppool,
        ):
            # ---- resident inputs -------------------------------------------------
            xT_sb = cpool.tile([128, KD, T], BF16)
            nc.sync.dma_start(out=xT_sb[:], in_=_r(xT[:]))
            w0a_sb = cpool.tile([128, KD, D], BF16)
            nc.sync.dma_start(out=w0a_sb[:], in_=_r(W0aT[:]))
            w1a_sb = cpool.tile([128, KD, H1], BF16)
            nc.sync.dma_start(out=w1a_sb[:], in_=_r(W1aT[:]))
            bh_sb = cpool.tile([1, OUT_HEAD], BF16)
            nc.sync.dma_start(out=bh_sb[:], in_=bh[:])
            ones_sb = cpool.tile([1, 128], BF16)
            nc.vector.memset(ones_sb[:], 1.0)

            h0T = cpool.tile([128, KD, T], BF16)  # (x @ W0a.T).T, hid on partitions
            h1T = cpool.tile([128, K1, T], BF16)  # (x @ W1a.T).T

            # ---- stats -----------------------------------------------------------
            t0_sums = spool.tile([128, MCH, 4], F32)
            t1_sums = spool.tile([128, MCH, n_t1b], F32)
            se_head = spool.tile([128, MCH], F32)
            se_t0 = spool.tile([128, MCH], F32)
            se_t1 = spool.tile([128, MCH], F32)
            lse_head = spool.tile([128, MCH], F32)
            lse_t0 = spool.tile([128, MCH], F32)
            lse_t1 = spool.tile([128, MCH], F32)
            c01 = spool.tile([128, MCH, 2], F32)  # head cluster logits (f32)
            neg_head = spool.tile([128, MCH], F32)
            tmp0 = spool.tile([128, MCH], F32)
            tmp1 = spool.tile([128, MCH], F32)
            neg0 = spool.tile([128, MCH], F32)
            neg1 = spool.tile([128, MCH], F32)

            def psum_tile():
                # one tag: [128, 2048] f32 = 4 PSUM banks; 2 bufs = all 8 banks
                return ppool.tile([128, 2048], F32, tag="ps", name="ps")

            # ---- phase H: hidden projections h0T / h1T ---------------------------
            for dst, wsb, nchunk in ((h0T, w0a_sb, KD), (h1T, w1a_sb, K1)):
                for hc in range(nchunk):
                    ps = psum_tile()
                    for k in range(KD):
                        nc.tensor.matmul(
                            ps[:, :T],
                            wsb[:, k, hc * 128 : (hc + 1) * 128],
                            xT_sb[:, k, :],
                            start=(k == 0),
                            stop=(k == KD - 1),
                        )
                    nc.vector.tensor_copy(dst[:, hc, :], ps[:, :T])

            # ---- HEAD + TAIL0: stream weights once, materialize fp16 logits ------
            head_logits = lpool.tile([128, MCH, OUT_HEAD], F16)
            t0_logits = lpool.tile([128, MCH, V0], F16)

            def mm_section(wdram, width, lhsT_sb, kt, visit, with_bias):
                """stream [128, kt, 1024] blocks of wdram; per (block, m, 512-vtile)
                run a psum accumulation group and visit(m, glob_off, vw, ps)."""
                for bo, bw in _blocks(width, 1024):
                    wb = wpool.tile([128, KD, 1024], BF16, tag="wblk")
                    nc.sync.dma_start(
                        out=wb[:, :kt, :bw], in_=_r(wdram[:])[:, :, bo : bo + bw]
                    )
                    for m in range(MCH):
                        ms = slice(m * 128, (m + 1) * 128)
                        for vo, vw in _blocks(bw, 512):
                            ps = psum_tile()
                            for k in range(kt):
                                nc.tensor.matmul(
                                    ps[:, :vw],
                                    lhsT_sb[:, k, ms],
                                    wb[:, k, vo : vo + vw],
                                    start=(k == 0),
                                    stop=(k == kt - 1 and not with_bias),
                                )
                            if with_bias:
                                nc.tensor.matmul(
                                    ps[:, :vw],
                                    ones_sb[:, :],
                                    bh_sb[:, bo + vo : bo + vo + vw],
                                    start=False,
                                    stop=True,
                                )
                            visit(m, bo + vo, vw, ps)

            def head_visit(m, go, vw, ps):
                nc.vector.tensor_copy(head_logits[:, m, go : go + vw], ps[:, :vw])
                if go + vw == OUT_HEAD:
                    nc.vector.tensor_copy(c01[:, m, :], ps[:, vw - 2 : vw])

            def t0_visit(m, go, vw, ps):
                nc.vector.tensor_copy(t0_logits[:, m, go : go + vw], ps[:, :vw])

            mm_section(WhT, OUT_HEAD, xT_sb, KD, head_visit, True)
            mm_section(W0bT, V0, h0T, KD, t0_visit, False)

            # head/t0 softmax stats from the materialized logits (wide ACT ops)
            for m in range(MCH):
                sc = scpool.tile([128, 2048], F16, tag="expsc")
                nc.scalar.activation(
                    sc[:, :OUT_HEAD],
                    head_logits[:, m, :],
                    AF.Exp,
                    accum_out=se_head[:, m : m + 1],
                )
                nc.scalar.activation(lse_head[:, m : m + 1], se_head[:, m : m + 1], AF.Ln)
                nc.vector.tensor_scalar_mul(
                    neg_head[:, m : m + 1], lse_head[:, m : m + 1], -1.0
                )
                for j, (so, sw) in enumerate(_blocks(V0, 2048)):
                    sc = scpool.tile([128, 2048], F16, tag="expsc")
                    nc.scalar.activation(
                        sc[:, :sw],
                        t0_logits[:, m, so : so + sw],
                        AF.Exp,
                        accum_out=t0_sums[:, m, j : j + 1],
                    )
                nc.vector.tensor_reduce(
                    se_t0[:, m : m + 1], t0_sums[:, m, :], X_AXIS, ALU.add
                )
                nc.scalar.activation(lse_t0[:, m : m + 1], se_t0[:, m : m + 1], AF.Ln)
                # bias = c0 - lse_head - lse_t0
                nc.vector.tensor_sub(
                    tmp0[:, m : m + 1], c01[:, m, 0:1], lse_head[:, m : m + 1]
                )
                nc.vector.tensor_sub(
                    neg0[:, m : m + 1], tmp0[:, m : m + 1], lse_t0[:, m : m + 1]
                )

            # head output rows: out[:, :2000] = head_logits - lse_head
            for m in range(MCH):
                ms = slice(m * 128, (m + 1) * 128)
                st = stpool.tile([128, 2048], F32, tag="stage")
                nc.scalar.activation(
                    st[:, :C0],
                    head_logits[:, m, :C0],
                    AF.Identity,
                    bias=neg_head[:, m : m + 1],
                )
                nc.sync.dma_start(out=out[ms, 0:C0], in_=st[:, :C0])

            # t0 output rows: out[:, 2000:10000] = t0_logits + (c0 - lse_head - lse_t0)
            for m in range(MCH):
                ms = slice(m * 128, (m + 1) * 128)
                for so, sw in _blocks(V0, 2048):
                    st = stpool.tile([128, 2048], F32, tag="stage")
                    nc.scalar.activation(
                        st[:, :sw],
                        t0_logits[:, m, so : so + sw],
                        AF.Identity,
                        bias=neg0[:, m : m + 1],
                    )
                    nc.sync.dma_start(
                        out=out[ms, T0_OFF + so : T0_OFF + so + sw], in_=st[:, :sw]
                    )

            # ---- TAIL1 pass 1: sumexp only (2048-wide psum groups) ---------------
            def t1_group(m, wb, bw, ps):
                ms = slice(m * 128, (m + 1) * 128)
                for vo, vw in _blocks(bw, 512):
                    for k in range(K1):
                        nc.tensor.matmul(
                            ps[:, vo : vo + vw],
                            h1T[:, k, ms],
                            wb[:, k, vo : vo + vw],
                            start=(k == 0),
                            stop=(k == K1 - 1),
                        )

            for bi, (bo, bw) in enumerate(t1_blocks):
                wb = w1pool.tile([128, K1, 2048], BF16, tag="wblk1")
                nc.sync.dma_start(
                    out=wb[:, :, :bw], in_=_r(W1bT[:])[:, :, bo : bo + bw]
                )
                for m in range(MCH):
                    ps = psum_tile()
                    t1_group(m, wb, bw, ps)
                    sc = scpool.tile([128, 2048], F16, tag="expsc")
                    nc.scalar.activation(
                        sc[:, :bw],
                        ps[:, :bw],
                        AF.Exp,
                        accum_out=t1_sums[:, m, bi : bi + 1],
                    )

            for m in range(MCH):
                nc.vector.tensor_reduce(
                    se_t1[:, m : m + 1], t1_sums[:, m, :], X_AXIS, ALU.add
                )
                nc.scalar.activation(lse_t1[:, m : m + 1], se_t1[:, m : m + 1], AF.Ln)
                # bias = c1 - lse_head - lse_t1
                nc.vector.tensor_sub(
                    tmp1[:, m : m + 1], c01[:, m, 1:2], lse_head[:, m : m + 1]
                )
                nc.vector.tensor_sub(
                    neg1[:, m : m + 1], tmp1[:, m : m + 1], lse_t1[:, m : m + 1]
                )

            # ---- TAIL1 pass 2: recompute logits, correct, write ------------------
            for bo, bw in t1_blocks:
                wb = w1pool.tile([128, K1, 2048], BF16, tag="wblk1")
                nc.sync.dma_start(
                    out=wb[:, :, :bw], in_=_r(W1bT[:])[:, :, bo : bo + bw]
                )
                for m in range(MCH):
                    ms = slice(m * 128, (m + 1) * 128)
                    ps = psum_tile()
                    t1_group(m, wb, bw, ps)
                    st = stpool.tile([128, 2048], F32, tag="stage")
                    nc.vector.tensor_scalar_add(
                        st[:, :bw], ps[:, :bw], neg1[:, m : m + 1]
                    )
                    nc.sync.dma_start(
                        out=out[ms, T1_OFF + bo : T1_OFF + bo + bw], in_=st[:, :bw]
                    )

    nc.compile()
    return nc


_NC_CACHE = {}


def _get_nc():
    if "nc" not in _NC_CACHE:
        _NC_CACHE["nc"] = build()
    return _NC_CACHE["nc"]


def _prep_weights(Wh, bh, W0a, W0b, W1a, W1b):
    f = ml_dtypes.bfloat16
    return {
        "WhT": np.ascontiguousarray(np.asarray(Wh, np.float32).T).astype(f),
        "bh": np.asarray(bh, np.float32).reshape(1, OUT_HEAD).astype(f),
        "W0aT": np.ascontiguousarray(np.asarray(W0a, np.float32).T).astype(f),
        "W0bT": np.ascontiguousarray(np.asarray(W0b, np.float32).T).astype(f),
        "W1aT": np.ascontiguousarray(np.asarray(W1a, np.float32).T).astype(f),
        "W1bT": np.ascontiguousarray(np.asarray(W1b, np.float32).T).astype(f),
    }


def kernel(x, Wh, bh, W0a, W0b, W1a, W1b, _trace=False):
    x = np.asarray(x, np.float32)
    nc = _get_nc()
    shared = _prep_weights(Wh, bh, W0a, W0b, W1a, W1b)
    in_maps = []
    for i in range(NCORES):
        m = dict(shared)
        m["xT"] = np.ascontiguousarray(x[i * T : (i + 1) * T].T).astype(
            ml_dtypes.bfloat16
        )
        in_maps.append(m)
    res = run_bass_kernel_spmd(nc, in_maps, core_ids=list(range(NCORES)), trace=_trace)
    out = np.concatenate([res.results[i]["out"] for i in range(NCORES)], axis=0)
    if _trace:
        return out, res
    return out
